# revision 16
# baseline (speedup 1.0000x reference)
"""TSM-style 3-tap depthwise temporal conv on 8 Trainium2 NeuronCores.

out[n, t, c, h, w] = w[c,0]*x[n,t-1,c,h,w] + w[c,1]*x[n,t,c,h,w]
                   + w[c,2]*x[n,t+1,c,h,w]   (zero-padded at clip edges)

Sharding: pure data parallel over the nt (clip-batch) axis — each of the 8
cores gets whole clips (nt=64, n_segment=8 -> one 8-frame clip per core).
Weight (c,3) is replicated.

Platform model (measured on this axon/trn2 virtualized stack): execution is
dominated by a large per-instruction dispatch cost (~40-60us plus a
size-dependent part), with limited engine/DMA overlap; standalone semaphore
instructions cost as much as compute ops. Design consequences:

  - raw bacc (nc.Block) instead of the Tile framework: every semaphore inc
    is attached to a data instruction via .then_inc and every DVE wait rides
    on a compute instruction via .wait_op (the Tile scheduler emits ~4
    standalone EventSemaphore instructions per pass, each costing a full
    dispatch). Note: an instruction holds at most ONE attached wait, and
    every DMA must carry a sem update or walrus crashes.
  - minimal instruction count (10 per pass) spread over ALL FOUR queues for
    overlap: gpsimd runs 2 casting loads (fp32 DRAM -> bf16 SBUF SWDGE,
    12.8MB read each) into SEPARATE x tiles so the block-B load overlaps
    block-A compute; ACT runs the two tensor-scalar muls (y = w1*x),
    halving DVE busy time; DVE runs only the 4 scalar_tensor_tensor
    accumulation taps; the otherwise-idle sync queue runs the 2 per-block
    bf16 stores, which keeps the steady-state cycle (compute -> store ->
    next load on the same buffers) per-block rather than whole-pass.
    (Merging the 2 loads into one 25.6MB DMA was tried and is ~2x WORSE:
    it destroys load/compute pipelining.)
  - bf16 x and y: rel err vs the fp32 reference is ~9e-3 (input rounding +
    3 output roundings), inside the 2e-2 gate; halves store bytes and
    speeds DVE accumulation.

Measured (k=2/122 repeat-chain differencing): 176-306us/pass depending on
device warm state, vs 697us for the Tile-framework fp32 baseline.
"""

import contextlib

import numpy as np

import concourse.bacc as bacc
import concourse.mybir as mybir
from concourse.bass_utils import run_bass_kernel_spmd

N_CORES = 8
P = 128  # SBUF partitions

FP = mybir.dt.float32
BF = mybir.dt.bfloat16
MULT = mybir.AluOpType.mult
ADD = mybir.AluOpType.add

_cache = {}


def emit_conv_raw(nc, w, sems, tiles, src, dst, F, C, n_seg, repeat=1,
                  chain=False):
    """Emit bodies for `repeat` conv passes src->dst (raw bacc, bf16 x/y).

    Returns (loads_body, stores_body, compute_body) closures for the
    gpsimd / scalar / vector engines. src/dst: callables k -> DRAM handle
    (src fp32, dst bf16). tiles: (wt, xA, xB, yb). chain=True adds the
    cross-pass waits used by the timing harness's scratch chain; the real
    kernel uses repeat=1, chain=False.
    """
    semF, semL, semC, semS, semT = sems
    wt, xA, xB, yb = tiles
    NB = C // P
    HW = xA.shape[2]
    n_clips = max(F // n_seg, 1)
    S = min(n_seg, F)
    xs = [xA, xB]
    ys = [yb[:, :, b, :] for b in range(NB)]

    def src_view(k, b):
        return src(k)[:, b * P:(b + 1) * P, :].rearrange("f c x -> c f x")

    def loads(eng):
        # gpsimd SWDGE: fp32 DRAM -> bf16 SBUF casting loads
        eng.dma_start(wt[:, :, :],
                      w.ap().rearrange("(b c) k -> c b k", c=P)).then_inc(semL, 16)
        if chain:
            eng.wait_ge(semF, 16 * 2 * 16 + 1)  # scratch fill done
        for k in range(repeat):
            for b in range(NB):
                if chain and k > 0:
                    # x[b] WAR: compute (k-1, b) consumed it. (The y WAR
                    # lives on the ACT mul, NOT here — keeping it off the
                    # loads lets next-pass loads stream without waiting for
                    # the previous pass's stores.)
                    eng.wait_ge(semC, NB * (k - 1) + b + 1)
                eng.dma_start(xs[b][:, :, :], src_view(k, b)).then_inc(semL, 16)

    def scalar_ops(eng):
        # ACT queue: only the two tensor_scalar muls (y = w1*x) per pass.
        # Offloading them halves DVE busy time.
        for k in range(repeat):
            for b in range(NB):
                i = NB * k + b
                if chain and k > 0:
                    # y[b] WAR: store (k-1, b) done reading this block's y
                    eng.wait_ge(semS, 16 * (NB * (k - 1) + b + 1))
                eng.wait_ge(semL, 16 * (i + 2))  # wtile + loads 0..i done
                eng.mul(ys[b], xs[b][:, :, :], wt[:, b, 1:2]).then_inc(semT, 1)

    def stores(eng):
        # sync queue (otherwise idle): per-block stores. Store b=0 runs
        # concurrently with block-1 compute; next-pass loads wait only
        # their own block's store.
        for k in range(repeat):
            for b in range(NB):
                eng.wait_ge(semC, NB * k + b + 1)
                eng.dma_start(
                    dst(k)[:, b * P:(b + 1) * P, :].rearrange("f c x -> c f x"),
                    ys[b]).then_inc(semS, 16)

    def compute(eng):
        # DVE: only the 4 accumulation taps; each block's first stt carries
        # the (single allowed) attached wait on the ACT mul for that block
        for k in range(repeat):
            for b in range(NB):
                i = NB * k + b
                w0 = wt[:, b, 0:1]
                w2 = wt[:, b, 2:3]
                y_ = ys[b]
                x_ = xs[b]
                first = True
                for c in range(n_clips):
                    lo, hi = c * S, (c + 1) * S
                    stt = eng.scalar_tensor_tensor(
                        y_[:, lo + 1:hi, :], x_[:, lo:hi - 1, :], w0,
                        y_[:, lo + 1:hi, :], MULT, ADD)
                    if first:
                        stt.wait_op(semT, i + 1, "sem-ge")
                        first = False
                    last = eng.scalar_tensor_tensor(
                        y_[:, lo:hi - 1, :], x_[:, lo + 1:hi, :], w2,
                        y_[:, lo:hi - 1, :], MULT, ADD)
                last.then_inc(semC, 1)

    return loads, scalar_ops, stores, compute


def _build(F, C, HW, n_seg):
    """Single-pass program: x (F, C, HW) f32 -> out (F, C, HW) bf16."""
    nc = bacc.Bacc("TRN2", target_bir_lowering=False, debug=False,
                   num_devices=N_CORES)
    x = nc.dram_tensor("x", (F, C, HW), FP, kind="ExternalInput")
    w = nc.dram_tensor("weight", (C, 3), FP, kind="ExternalInput")
    out = nc.dram_tensor("out", (F, C, HW), BF, kind="ExternalOutput")
    NB = C // P

    stack = contextlib.ExitStack()
    block = stack.enter_context(nc.Block())
    sems = tuple(stack.enter_context(nc.semaphore(s))
                 for s in ("semF", "semL", "semC", "semS", "semT"))
    wt = stack.enter_context(nc.sbuf_tensor("wt", [P, NB, 3], FP))
    xA = stack.enter_context(nc.sbuf_tensor("xA", [P, F, HW], BF))
    xB = stack.enter_context(nc.sbuf_tensor("xB", [P, F, HW], BF))
    yb = stack.enter_context(nc.sbuf_tensor("yb", [P, F, NB, HW], BF))

    loads, scalar_ops, stores, compute = emit_conv_raw(
        nc, w, sems, (wt, xA, xB, yb), lambda k: x, lambda k: out,
        F, C, n_seg, repeat=1, chain=False)

    block.gpsimd(loads)
    block.scalar(scalar_ops)
    block.sync(stores)
    block.vector(compute)
    stack.close()
    nc.compile()
    return nc


def _get_program(F, C, HW, n_seg):
    key = (F, C, HW, n_seg)
    if key not in _cache:
        _cache[key] = _build(F, C, HW, n_seg)
    return _cache[key]


def kernel(x, weight, n_segment, **_kw):
    x = np.asarray(x)
    weight = np.ascontiguousarray(np.asarray(weight, dtype=np.float32))
    n_seg = int(np.asarray(n_segment))
    nt, C, H, W = x.shape
    HW = H * W
    assert nt % N_CORES == 0
    F = nt // N_CORES
    # each core must hold whole clips
    assert F % n_seg == 0 or n_seg % F == 0, (F, n_seg)
    assert C % P == 0, C

    nc = _get_program(F, C, HW, n_seg)

    xs = np.ascontiguousarray(x, dtype=np.float32).reshape(nt, C, HW)
    in_maps = [
        {"x": xs[i * F:(i + 1) * F], "weight": weight} for i in range(N_CORES)
    ]
    res = run_bass_kernel_spmd(nc, in_maps, list(range(N_CORES)))
    out = np.concatenate(
        [np.asarray(res.results[i]["out"], dtype=np.float32)
         for i in range(N_CORES)], axis=0)
    return out.reshape(nt, C, H, W).astype(np.float32, copy=False)
